# revision 3
# baseline (speedup 1.0000x reference)
"""Trainium2 Bass kernel for nn_DecLayer (gnn_message_passing).

B, N, K, H, NI = 8, 4096, 32, 128, 384.  Data-parallel over batch: core b
processes batch element b (4096 nodes, 131072 edges, 201MB of h_E).

Per-core dataflow (per 512-edge tile, 256 tiles):
  DMA h_E tile [512e, 384] -> SBUF [128p, 4eb, 384]
  PE transposes (12x 128x128, f32r) -> PSUM -> ACT evac -> hE^T [NI, e]
  z1 = sum_c W1e_c^T.T @ hET_c + W1v^T.T @ hv_bcast          (PSUM)
  m1 = gelu(z1 + b1)                                          (ACT)
  z2 = W2^T.T @ m1 + (-BIG) x (1-mask)   rank-1 inject        (PSUM)
  m2 = gelu(z2 + b2)      -> masked edge columns are exactly 0
  s[:, nodes] = grouped-reduce_k(m2)                          (DVE)
Then a node-level phase: dh = (W3@s + b3*c)/SCALE, LN1, FFN, LN2, mask_V,
transpose back and DMA out.  All matmuls f32r (tf32) except the W3 group
and final transposes (fp32).
"""
import sys
import numpy as np
from contextlib import ExitStack

sys.path.insert(0, "/opt/trn_rl_repo")
import concourse.bacc as bacc
import concourse.tile as tile
from concourse import mybir
from concourse.bass_utils import run_bass_kernel_spmd

F32 = mybir.dt.float32
F32R = mybir.dt.float32r
AF = mybir.ActivationFunctionType
ALU = mybir.AluOpType
AX = mybir.AxisListType

B, N, K, H, NI = 8, 4096, 32, 128, 384
SCALE = 30.0
EPS = 1e-5
BIG = 1.0e5

E_TILE = 512            # edges per phase-1 tile (= 16 nodes)
NT = (N * K) // E_TILE  # 256 phase-1 tiles
N_TILE = 512            # nodes per phase-2 tile
FH = 4 * H              # 512

# const layout (f32r [128, C_END])
C_ID = 0          # identity [128,128]
C_W1E = 128       # W1e^T 3 chunks [384->3x128, 128]
C_W1V = 512       # W1v^T
C_W2 = 640        # W2^T
C_W3 = 768        # (W3/SCALE)^T   (used as fp32 via bitcast)
C_WIN = 896       # Win^T [128, 512]
C_WOUT = 1408     # Wout^T 4 chunks [128,128]
C_ONESC = 1920    # ones column [128,1]
C_NEG = 1921      # row0 = -BIG      [1,128]
C_B3 = 2049       # row0 = W3_b/SCALE [1,128]
C_ONESR = 2177    # row0 = ones      [1,128]
C_END = 2305

# f32 bias columns
BC_B1, BC_B2, BC_BIN, BC_BOUT, BC_G1, BC_BL1, BC_G2, BC_BL2 = 0, 1, 2, 6, 7, 8, 9, 10
BC_EPS = 11
BC_END = 12

_NC_CACHE = {}


def _build_nc():
    nc = bacc.Bacc(trn_type="TRN2")
    he = nc.dram_tensor("he", [N * K, NI], F32R, kind="ExternalInput")
    hv = nc.dram_tensor("hv", [N, H], F32, kind="ExternalInput")
    mkc = nc.dram_tensor("mkc", [1, N * K], F32R, kind="ExternalInput")
    crow = nc.dram_tensor("crow", [1, N], F32R, kind="ExternalInput")
    mvrow = nc.dram_tensor("mvrow", [1, N], F32R, kind="ExternalInput")
    cst = nc.dram_tensor("cst", [128, C_END], F32R, kind="ExternalInput")
    bcol = nc.dram_tensor("bcol", [128, BC_END], F32, kind="ExternalInput")
    out = nc.dram_tensor("out", [N, H], F32, kind="ExternalOutput")

    with ExitStack() as ctx:
        tc = ctx.enter_context(tile.TileContext(nc))
        # long-lived buffers
        glob = ctx.enter_context(tc.tile_pool(name="glob", bufs=1))
        cst_t = glob.tile([128, C_END], F32R)
        bcol_t = glob.tile([128, BC_END], F32)
        hvt_r = glob.tile([128, N], F32R)   # h_V^T for phase 1
        hvt_f = glob.tile([128, N], F32)    # h_V^T full precision for phase 2
        s_buf = glob.tile([128, N], F32)    # masked K-sums per node
        crow_t = glob.tile([1, N], F32R)
        mvrow_t = glob.tile([1, N], F32R)

        nc.sync.dma_start(cst_t[:], cst[:])
        nc.sync.dma_start(bcol_t[:], bcol[:])
        nc.sync.dma_start(crow_t[:], crow[:])
        nc.sync.dma_start(mvrow_t[:], mvrow[:])

        def cs(a, b):
            return cst_t[:, a:b]

        id_r = cs(C_ID, C_ID + 128)
        id_f = id_r.bitcast(F32)
        w1e = [cs(C_W1E + c * 128, C_W1E + (c + 1) * 128) for c in range(3)]
        w1v = cs(C_W1V, C_W1V + 128)
        w2 = cs(C_W2, C_W2 + 128)
        w3_f = cs(C_W3, C_W3 + 128).bitcast(F32)
        win = [cs(C_WIN + q * 128, C_WIN + (q + 1) * 128) for q in range(4)]
        wout = [cs(C_WOUT + q * 128, C_WOUT + (q + 1) * 128) for q in range(4)]
        ones_c = cs(C_ONESC, C_ONESC + 1)
        neg_r = cst_t[0:1, C_NEG:C_NEG + 128]
        b3_rf = cst_t[0:1, C_B3:C_B3 + 128].bitcast(F32)
        ones_r = cst_t[0:1, C_ONESR:C_ONESR + 128]
        bc = lambda i: bcol_t[:, i:i + 1]

        # ---------------- phase 0: transpose h_V ----------------
        with ExitStack() as p0:
            p0sb = p0.enter_context(tc.tile_pool(name="p0sb", bufs=2))
            p0ps = p0.enter_context(tc.tile_pool(name="p0ps", bufs=2, space="PSUM"))
            hv_nat = p0sb.tile([128, N // 128, 128], F32, tag="hvnat")
            nc.sync.dma_start(hv_nat[:], hv[:].rearrange("(g p) h -> p g h", p=128))
            for grp in range(N // 512):
                pt0 = p0ps.tile([128, 512], F32, tag="pt0")
                for j in range(4):
                    g = grp * 4 + j
                    nc.tensor.transpose(pt0[:, j * 128:(j + 1) * 128],
                                        hv_nat[:, g, :], id_f)
                seg = slice(grp * 512, (grp + 1) * 512)
                nc.scalar.activation(hvt_r[:, seg], pt0[:], AF.Copy)
                nc.scalar.activation(hvt_f[:, seg], pt0[:], AF.Copy)

        # ---------------- phase 1: edge tiles ----------------
        with ExitStack() as p1:
            dpool = p1.enter_context(tc.tile_pool(name="dpool", bufs=3))
            mpool = p1.enter_context(tc.tile_pool(name="mpool", bufs=4))
            hpool = p1.enter_context(tc.tile_pool(name="hpool", bufs=2))
            apool = p1.enter_context(tc.tile_pool(name="apool", bufs=2))
            ps_t = p1.enter_context(tc.tile_pool(name="ps_t", bufs=4, space="PSUM"))
            ps_z1 = p1.enter_context(tc.tile_pool(name="ps_z1", bufs=2, space="PSUM"))
            ps_z2 = p1.enter_context(tc.tile_pool(name="ps_z2", bufs=2, space="PSUM"))

            for t in range(NT):
                e0 = t * E_TILE
                n0 = t * (E_TILE // K)  # 16 nodes per tile
                henat = dpool.tile([128, 4, NI], F32R, tag="henat")
                nc.sync.dma_start(
                    henat[:],
                    he[e0:e0 + E_TILE, :].rearrange("(eb p) ni -> p eb ni", p=128))
                mkc_t = mpool.tile([1, E_TILE], F32R, tag="mkc")
                nc.sync.dma_start(mkc_t[:], mkc[0:1, e0:e0 + E_TILE])

                het = hpool.tile([128, 3 * E_TILE], F32R, tag="het")
                for c in range(3):
                    pt = ps_t.tile([128, E_TILE], F32R, tag="pt")
                    for eb in range(4):
                        nc.tensor.transpose(
                            pt[:, eb * 128:(eb + 1) * 128],
                            henat[:, eb, c * 128:(c + 1) * 128], id_r)
                    nc.scalar.activation(
                        het[:, c * E_TILE:(c + 1) * E_TILE], pt[:], AF.Copy)

                z1 = ps_z1.tile([128, E_TILE], F32, tag="z1")
                for c in range(3):
                    nc.tensor.matmul(z1[:], w1e[c],
                                     het[:, c * E_TILE:(c + 1) * E_TILE],
                                     start=(c == 0), stop=False)
                hv_b = hvt_r[:, n0:n0 + 16].to_broadcast([128, 16, K])
                nc.tensor.matmul(z1[:], w1v, hv_b, start=False, stop=True)
                m1 = apool.tile([128, E_TILE], F32R, tag="m1")
                nc.scalar.activation(m1[:], z1[:], AF.Gelu, bias=bc(BC_B1))

                z2 = ps_z2.tile([128, E_TILE], F32, tag="z2")
                nc.tensor.matmul(z2[:], w2, m1[:], start=True, stop=False)
                nc.tensor.matmul(z2[:], neg_r, mkc_t[:], start=False, stop=True)
                m2 = apool.tile([128, E_TILE], F32R, tag="m2")
                nc.scalar.activation(m2[:], z2[:], AF.Gelu, bias=bc(BC_B2))

                nc.vector.tensor_reduce(
                    s_buf[:, n0:n0 + 16],
                    m2[:].rearrange("p (n k) -> p n k", k=K),
                    op=ALU.add, axis=AX.X)

        # ---------------- phase 2: node tiles ----------------
        with ExitStack() as p2:
            sb2 = p2.enter_context(tc.tile_pool(name="sb2", bufs=2))
            ps_mm = p2.enter_context(tc.tile_pool(name="ps_mm", bufs=2, space="PSUM"))
            ps_bc = p2.enter_context(tc.tile_pool(name="ps_bc", bufs=2, space="PSUM"))
            ps_ms = p2.enter_context(tc.tile_pool(name="ps_ms", bufs=2, space="PSUM"))
            ps_ff = p2.enter_context(tc.tile_pool(name="ps_ff", bufs=2, space="PSUM"))

            def layer_norm(x_in, g_ap, b_ap, out_dtype, tag):
                """x_in: [128, N_TILE] f32r SBUF -> normalized [128, N_TILE]."""
                sq = sb2.tile([128, N_TILE], F32R, tag=tag + "sq")
                nc.scalar.activation(sq[:], x_in[:], AF.Square)
                s1 = ps_ms.tile([1, N_TILE], F32, tag="ms")
                nc.tensor.matmul(s1[:], ones_c, x_in[:], start=True, stop=True)
                s2 = ps_ms.tile([1, N_TILE], F32, tag="ms")
                nc.tensor.matmul(s2[:], ones_c, sq[:], start=True, stop=True)
                mu = sb2.tile([1, N_TILE], F32R, tag=tag + "mu")
                nc.scalar.activation(mu[:], s1[:], AF.Copy, scale=1.0 / 128)
                s2r = sb2.tile([1, N_TILE], F32, tag=tag + "s2r")
                nc.scalar.activation(s2r[:], s2[:], AF.Copy, scale=1.0 / 128)
                musq = sb2.tile([1, N_TILE], F32, tag=tag + "musq")
                nc.vector.tensor_tensor(musq[:], mu[:].bitcast(F32), mu[:].bitcast(F32),
                                        op=ALU.mult)
                var = sb2.tile([1, N_TILE], F32, tag=tag + "var")
                nc.vector.tensor_tensor(var[:], s2r[:], musq[:], op=ALU.subtract)
                sd = sb2.tile([1, N_TILE], F32R, tag=tag + "sd")
                nc.scalar.activation(sd[:], var[:], AF.Sqrt,
                                     bias=bcol_t[0:1, BC_EPS:BC_EPS + 1])
                mu_b = ps_bc.tile([128, N_TILE], F32, tag="bc")
                nc.tensor.matmul(mu_b[:], ones_r, mu[:], start=True, stop=True)
                sd_b = ps_bc.tile([128, N_TILE], F32, tag="bc")
                nc.tensor.matmul(sd_b[:], ones_r, sd[:], start=True, stop=True)
                d = sb2.tile([128, N_TILE], F32, tag=tag + "d")
                nc.vector.tensor_tensor(d[:], x_in[:].bitcast(F32), mu_b[:],
                                        op=ALU.subtract)
                rec = sb2.tile([128, N_TILE], F32, tag=tag + "rec")
                nc.vector.reciprocal_approx_fast(rec[:], sd_b[:])
                u = sb2.tile([128, N_TILE], F32, tag=tag + "u")
                nc.vector.tensor_tensor(u[:], d[:], rec[:], op=ALU.mult)
                y = sb2.tile([128, N_TILE], out_dtype, tag=tag + "y")
                nc.scalar.activation(y[:], u[:], AF.Identity, scale=g_ap, bias=b_ap)
                return y

            for t in range(N // N_TILE):
                n0 = t * N_TILE
                seg = slice(n0, n0 + N_TILE)
                # dh group (fp32): W3'@s + b3'*c + h_V
                zp = ps_mm.tile([128, N_TILE], F32, tag="mm")
                nc.tensor.matmul(zp[:], w3_f, s_buf[:, seg], start=True, stop=False)
                nc.tensor.matmul(zp[:], b3_rf, crow_t[0:1, seg].bitcast(F32),
                                 start=False, stop=False)
                nc.tensor.matmul(zp[:], id_f, hvt_f[:, seg], start=False, stop=True)
                x1 = sb2.tile([128, N_TILE], F32R, tag="x1")
                nc.scalar.activation(x1[:], zp[:], AF.Copy)

                y1 = layer_norm(x1, bc(BC_G1), bc(BC_BL1), F32R, "L1")

                # FFN
                ffq = sb2.tile([128, 4, N_TILE], F32R, tag="ffq")
                for q in range(4):
                    f1 = ps_ff.tile([128, N_TILE], F32, tag="f1")
                    nc.tensor.matmul(f1[:], win[q], y1[:], start=True, stop=True)
                    nc.scalar.activation(ffq[:, q, :], f1[:], AF.Gelu,
                                         bias=bcol_t[:, BC_BIN + q:BC_BIN + q + 1])
                z4 = ps_mm.tile([128, N_TILE], F32, tag="mm")
                for q in range(4):
                    nc.tensor.matmul(z4[:], wout[q], ffq[:, q, :],
                                     start=(q == 0), stop=False)
                nc.tensor.matmul(z4[:], id_r, y1[:], start=False, stop=True)
                x2 = sb2.tile([128, N_TILE], F32R, tag="x2")
                nc.scalar.activation(x2[:], z4[:], AF.Identity, bias=bc(BC_BOUT))

                y2 = layer_norm(x2, bc(BC_G2), bc(BC_BL2), F32, "L2")

                mv_b = ps_bc.tile([128, N_TILE], F32, tag="bc")
                nc.tensor.matmul(mv_b[:], ones_r, mvrow_t[0:1, seg],
                                 start=True, stop=True)
                y2m = sb2.tile([128, N_TILE], F32, tag="y2m")
                nc.vector.tensor_tensor(y2m[:], y2[:], mv_b[:], op=ALU.mult)

                # transpose back to [node, H] and store
                yt = ps_ms.tile([128, N_TILE], F32, tag="ms")
                for j in range(4):
                    nc.tensor.transpose(yt[:, j * 128:(j + 1) * 128],
                                        y2m[:, j * 128:(j + 1) * 128], id_f)
                osb = sb2.tile([128, 4, 128], F32, tag="osb")
                nc.scalar.activation(osb[:].rearrange("p a b -> p (a b)"), yt[:],
                                     AF.Copy)
                nc.sync.dma_start(
                    out[n0:n0 + N_TILE, :].rearrange("(nb p) h -> p nb h", p=128),
                    osb[:])

    nc.compile()
    return nc


def _prep_consts(W1_w, W1_b, W2_w, W2_b, W3_w, W3_b,
                 ln1_g, ln1_b, ln2_g, ln2_b, Win_w, Win_b, Wout_w, Wout_b):
    cst = np.zeros((128, C_END), np.float32)
    cst[:, C_ID:C_ID + 128] = np.eye(128)
    w1eT = W1_w[:, H:].T  # [384, 128]
    for c in range(3):
        cst[:, C_W1E + c * 128:C_W1E + (c + 1) * 128] = w1eT[c * 128:(c + 1) * 128]
    cst[:, C_W1V:C_W1V + 128] = W1_w[:, :H].T
    cst[:, C_W2:C_W2 + 128] = W2_w.T
    cst[:, C_W3:C_W3 + 128] = (W3_w / SCALE).T
    cst[:, C_WIN:C_WIN + FH] = Win_w.T
    woutT = Wout_w.T  # [512, 128]
    for q in range(4):
        cst[:, C_WOUT + q * 128:C_WOUT + (q + 1) * 128] = \
            woutT[q * 128:(q + 1) * 128]
    cst[:, C_ONESC] = 1.0
    cst[0, C_NEG:C_NEG + 128] = -BIG
    cst[0, C_B3:C_B3 + 128] = W3_b / SCALE
    cst[0, C_ONESR:C_ONESR + 128] = 1.0

    bcol = np.zeros((128, BC_END), np.float32)
    bcol[:, BC_B1] = W1_b
    bcol[:, BC_B2] = W2_b
    for q in range(4):
        bcol[:, BC_BIN + q] = Win_b[q * 128:(q + 1) * 128]
    bcol[:, BC_BOUT] = Wout_b
    bcol[:, BC_G1] = ln1_g
    bcol[:, BC_BL1] = ln1_b
    bcol[:, BC_G2] = ln2_g
    bcol[:, BC_BL2] = ln2_b
    bcol[:, BC_EPS] = EPS
    return cst, bcol


def kernel(h_V, h_E, mask_V, mask_attend,
           W1_w, W1_b, W2_w, W2_b, W3_w, W3_b,
           ln1_g, ln1_b, ln2_g, ln2_b,
           Win_w, Win_b, Wout_w, Wout_b, _trace=False):
    h_V = np.asarray(h_V, np.float32)
    h_E = np.asarray(h_E, np.float32)
    mask_V = np.asarray(mask_V, np.float32)
    mask_attend = np.asarray(mask_attend, np.float32)
    args = [np.asarray(a, np.float32) for a in
            (W1_w, W1_b, W2_w, W2_b, W3_w, W3_b,
             ln1_g, ln1_b, ln2_g, ln2_b, Win_w, Win_b, Wout_w, Wout_b)]
    cst, bcol = _prep_consts(*args)

    if "nc" not in _NC_CACHE:
        _NC_CACHE["nc"] = _build_nc()
    nc = _NC_CACHE["nc"]

    maskc = (1.0 - mask_attend).reshape(B, 1, N * K)
    crow = mask_attend.sum(-1).reshape(B, 1, N)
    in_maps = []
    for b in range(B):
        in_maps.append(dict(
            he=h_E[b].reshape(N * K, NI),
            hv=h_V[b],
            mkc=maskc[b],
            crow=crow[b],
            mvrow=mask_V[b].reshape(1, N),
            cst=cst, bcol=bcol))

    res = run_bass_kernel_spmd(nc, in_maps, core_ids=list(range(B)),
                               trace=_trace)
    out = np.stack([res.results[b]["out"] for b in range(B)])
    if _trace:
        return out, res
    return out
